# revision 4
# baseline (speedup 1.0000x reference)
"""Trainium2 Bass kernel for nn_CovarianceEstimator.

Computes, for y [B=16, R=1, A=16, T=14, S=1024] complex (given as separate
real/imag f32 tensors):
  - gather P=1024 pilot positions (sym_p, sc_p) from estimation_indices
  - per-position A x A outer products sig_p sig_p^H
  - unsorted-segment-mean over subcarrier ids sc_p
  - nearest-neighbor expand via closest_subcarrier to all S subcarriers
  - broadcast over T symbols
Output: [B, R, T, S, A, A] complex64.

Sharding: data-parallel over batch; 2 batches per core on 8 cores.

Two device-program builders:
  * fast path  - used when the index tensors match the PilotPattern structure
                 (meshgrid of 2 symbols x every-2nd-subcarrier, closest = even
                 floor).  Pure DVE + DMA, exact f32 math.
  * generic    - any estimation_indices / closest_subcarrier.  Host folds the
                 whole segment-mean + NN-gather into one dense [S, P] weight
                 matrix applied on the tensor engine.
"""

import numpy as np

B, R, A, T, S = 16, 1, 16, 14, 1024
P_EST = 1024          # number of (sym, sc) estimation positions
N_CORES = 8
B_LOC = B // N_CORES  # 2 batches per core
AA2 = A * A * 2       # interleaved (re, im) row payload per subcarrier

_cache = {}


def _fast_path_info(est, closest):
    """Return (sym0, sym1) if indices match the pilot-pattern structure:
    est == meshgrid([sym0, sym1], arange(0, S, 2)) row-major and
    closest == 2*(arange(S)//2).  Else None."""
    if est.shape != (P_EST, 2) or closest.shape != (S,):
        return None
    sc = np.arange(0, S, 2, dtype=est.dtype)
    if not np.array_equal(est[: S // 2, 1], sc):
        return None
    if not np.array_equal(est[S // 2 :, 1], sc):
        return None
    sym0 = int(est[0, 0])
    sym1 = int(est[S // 2, 0])
    if not (0 <= sym0 < T and 0 <= sym1 < T):
        return None
    if not np.all(est[: S // 2, 0] == sym0):
        return None
    if not np.all(est[S // 2 :, 0] == sym1):
        return None
    if not np.array_equal(closest, (2 * (np.arange(S) // 2)).astype(closest.dtype)):
        return None
    return sym0, sym1


def _build_fast(sym0, sym1):
    """DVE-only program.  Per batch:
      - DMA the two pilot-symbol slabs y[b,:,sym,:] into SBUF laid out
        [q, a, k] with subcarrier s = q*8 + k  (contiguous innermost runs).
      - strided on-chip copy selects even subcarriers: sig_h[q, m, a] for
        pair index s' = q*4 + m  (sc = 2*s'), scaled by sqrt(1/2) so every
        product carries the segment-mean 1/2.
      - DVE outer products + pair sums -> f[q, m, i*A+j, re/im]
      - duplicate rows (nearest-neighbor expand) into fd so each partition
        holds output rows s = q*8 .. q*8+7 contiguously, then one plain
        [128, 4096] DMA per (b, t)."""
    import concourse.bacc as bacc
    import concourse.mybir as mybir
    from concourse.tile import TileContext

    f32 = mybir.dt.float32
    bf16 = mybir.dt.bfloat16
    nc = bacc.Bacc(trn_type="TRN2", target_bir_lowering=False)
    yr = nc.declare_dram_parameter("yr", [B_LOC, A, T, S], f32, isOutput=False)
    yi = nc.declare_dram_parameter("yi", [B_LOC, A, T, S], f32, isOutput=False)
    out = nc.declare_dram_parameter("out", [B_LOC, T, S, AA2], f32, isOutput=True)

    KS = S // 128  # 8 subcarriers per partition
    M = KS // 2    # 4 subcarrier-pairs per partition

    with TileContext(nc) as tc:
        with (
            tc.tile_pool(name="slab", bufs=2) as slabp,
            tc.tile_pool(name="sig", bufs=2) as sigp,
            tc.tile_pool(name="g", bufs=2) as gp,
            tc.tile_pool(name="f", bufs=2) as fp,
        ):
            for b in range(B_LOC):
                # per-pilot-symbol slab loads: [q, a, k] with s = q*8+k.
                # Small (64KB) so the serial prefix before compute is short;
                # spread across the two DGE paths.
                sr = sigp.tile([128, 2, M, A], f32, tag="sr")  # [q, h, m, a]
                si = sigp.tile([128, 2, M, A], f32, tag="si")
                for part, (ysrc, dst, eng) in enumerate(
                    ((yr, sr, nc.scalar), (yi, si, nc.gpsimd))
                ):
                    for h, sym in enumerate((sym0, sym1)):
                        slab = slabp.tile([128, A, KS], f32, tag=f"slab{part}{h}")
                        eng.dma_start(
                            out=slab[:],
                            in_=ysrc[b, :, sym, :].rearrange(
                                "a (q k) -> q a k", q=128, k=KS
                            ),
                        )
                        # select even subcarriers, transpose (a,k)->(m,a),
                        # scale by sqrt(1/2) for the segment mean
                        nc.vector.tensor_scalar_mul(
                            dst[:, h],
                            slab[:, :, 0:KS:2].transpose([0, 2, 1]),
                            0.7071067811865476,
                        )

                HM = 2 * M  # merged (h, m) dim

                def vi(x):  # varies over i, broadcast over j; h merged in
                    return (
                        x[:]
                        .rearrange("q h m a -> q (h m) a")[:, :, :, None]
                        .to_broadcast([128, HM, A, A])
                    )

                def vj(x):  # broadcast over i, varies over j
                    return (
                        x[:]
                        .rearrange("q h m a -> q (h m) a")[:, :, None, :]
                        .to_broadcast([128, HM, A, A])
                    )

                # fd[q, m, e, i*A+j, re/im]: output rows s = q*8 + m*2 + e
                # bf16: halves the SBUF-fabric bytes of the output DMA; the
                # SWDGE cast-DMA upconverts to f32 on the way to HBM.
                fd = fp.tile([128, M, 2, A * A, 2], bf16, tag="fd")
                u0 = gp.tile([128, HM, A, A], f32, tag="u0")
                u1 = gp.tile([128, HM, A, A], f32, tag="u1")
                v0 = gp.tile([128, M, A, A], f32, tag="v0")
                v1 = gp.tile([128, M, A, A], f32, tag="v1")
                # real: sum_h SrSr + SiSi, written to both e slots
                nc.vector.tensor_mul(u0[:], vi(sr), vj(sr))
                nc.vector.tensor_mul(u1[:], vi(si), vj(si))
                nc.vector.tensor_add(v0[:], u0[:, :M], u0[:, M:])
                nc.vector.tensor_add(v1[:], u1[:, :M], u1[:, M:])
                nc.vector.tensor_add(fd[:, :, 0, :, 0], v0[:], v1[:])
                nc.vector.tensor_add(fd[:, :, 1, :, 0], v0[:], v1[:])
                # imag: sum_h SiSr - SrSi
                nc.vector.tensor_mul(u0[:], vi(si), vj(sr))
                nc.vector.tensor_mul(u1[:], vi(sr), vj(si))
                nc.vector.tensor_add(v0[:], u0[:, :M], u0[:, M:])
                nc.vector.tensor_add(v1[:], u1[:, :M], u1[:, M:])
                nc.vector.tensor_sub(fd[:, :, 0, :, 1], v0[:], v1[:])
                nc.vector.tensor_sub(fd[:, :, 1, :, 1], v0[:], v1[:])

                # --- output rows s = q*8 + (m*2+e), contiguous per partition.
                # One fused DMA per batch covers all T symbol copies via a
                # stride-0 t dim on the SBUF source.  The walrus DIRECT2D DMA
                # form accepts only ONE sync wait; with slab loads on SWDGE
                # lanes, each batch's single output DMA lands on a fresh HWDGE
                # lane and needs only the fd-ready wait.
                src = (
                    fd[:]
                    .rearrange("q m e c ri -> q (m e c ri)")[:, None, :]
                    .to_broadcast([128, T, M * 2 * A * A * 2])
                )
                dst = out[b].rearrange("t (q k) c -> q t (k c)", q=128, k=KS)
                nc.gpsimd.dma_start(out=dst, in_=src)
    nc.finalize()
    return nc


def _build_generic(est, closest):
    """Generic program: host-gathered sig^T comes in as an input; the whole
    segment-mean + NN-gather is one dense weight matmul on the PE.
      cov[s, (i,j)] = sum_p wt[p, s] * G[p, (i,j)],  G from sig outer products.
    """
    import concourse.bacc as bacc
    import concourse.mybir as mybir
    from concourse.tile import TileContext

    f32 = mybir.dt.float32
    nc = bacc.Bacc(trn_type="TRN2", target_bir_lowering=False)
    # sig^T per batch: [P_EST, A] split as [KP=8, 128, A]
    sgr = nc.declare_dram_parameter("sgr", [B_LOC, P_EST // 128, 128, A], f32, isOutput=False)
    sgi = nc.declare_dram_parameter("sgi", [B_LOC, P_EST // 128, 128, A], f32, isOutput=False)
    wt = nc.declare_dram_parameter("wt", [P_EST, S], f32, isOutput=False)
    out = nc.declare_dram_parameter("out", [B_LOC, T, S, AA2], f32, isOutput=True)

    KP = P_EST // 128  # contraction chunks
    MS = S // 128      # output subcarrier chunks

    with TileContext(nc) as tc:
        with (
            tc.tile_pool(name="w", bufs=1) as wp,
            tc.tile_pool(name="sig", bufs=2) as sigp,
            tc.tile_pool(name="g", bufs=4) as gp,
            tc.tile_pool(name="ps", bufs=8, space="PSUM") as psp,
            tc.tile_pool(name="f", bufs=2) as fp,
        ):
            w_all = wp.tile([128, KP, S], f32, name="w_all")
            nc.sync.dma_start(
                out=w_all[:], in_=wt[:].rearrange("(k q) s -> q k s", k=KP, q=128)
            )
            for b in range(B_LOC):
                sr = sigp.tile([128, KP, A], f32, tag="sr")
                si = sigp.tile([128, KP, A], f32, tag="si")
                nc.sync.dma_start(
                    out=sr[:], in_=sgr[b].rearrange("k q a -> q k a")
                )
                nc.sync.dma_start(
                    out=si[:], in_=sgi[b].rearrange("k q a -> q k a")
                )

                f = fp.tile([128, MS, A * A, 2], f32, tag="f")
                gtiles = {}
                for k in range(KP):
                    def ii(x):
                        return x[:, k, :, None].to_broadcast([128, A, A])

                    def jj(x):
                        return x[:, k, None, :].to_broadcast([128, A, A])

                    gr = gp.tile([128, A, A], f32, tag=f"gr{k}")
                    gi = gp.tile([128, A, A], f32, tag=f"gi{k}")
                    tt = gp.tile([128, A, A], f32, tag="tt")
                    nc.vector.tensor_mul(gr[:], ii(sr), jj(sr))
                    nc.vector.tensor_mul(tt[:], ii(si), jj(si))
                    nc.vector.tensor_add(gr[:], gr[:], tt[:])
                    nc.vector.tensor_mul(gi[:], ii(si), jj(sr))
                    nc.vector.tensor_mul(tt[:], ii(sr), jj(si))
                    nc.vector.tensor_sub(gi[:], gi[:], tt[:])
                    gtiles[k] = (gr, gi)

                for m in range(MS):
                    for part in range(2):
                        pp = psp.tile([128, A * A], f32, tag="pp")
                        for k in range(KP):
                            g = gtiles[k][part]
                            nc.tensor.matmul(
                                pp[:],
                                lhsT=w_all[:, k, m * 128 : (m + 1) * 128],
                                rhs=g[:].rearrange("q i j -> q (i j)"),
                                start=(k == 0),
                                stop=(k == KP - 1),
                            )
                        nc.vector.tensor_copy(f[:, m, :, part], pp[:])

                src = f[:]
                for t in range(T):
                    dst = out[b, t].rearrange(
                        "(m q) (ij ri) -> q m ij ri", m=MS, q=128, ij=A * A, ri=2
                    )
                    nc.sync.dma_start(out=dst, in_=src)
    nc.finalize()
    return nc


def _get_program(est, closest):
    key = (est.tobytes(), closest.tobytes())
    hit = _cache.get(key)
    if hit is not None:
        return hit
    fast = _fast_path_info(est, closest)
    if fast is not None:
        prog = ("fast", _build_fast(*fast), None)
    else:
        counts = np.zeros(S, dtype=np.float64)
        np.add.at(counts, est[:, 1], 1.0)
        denom = np.maximum(counts, 1.0)
        # wt[p, s] = [sc_p == closest[s]] / denom[closest[s]]
        wt = (
            (est[:, 1][:, None] == closest[None, :]).astype(np.float32)
            / denom[closest][None, :].astype(np.float32)
        )
        prog = ("generic", _build_generic(est, closest), np.ascontiguousarray(wt))
    _cache[key] = prog
    return prog


def kernel(y_real, y_imag, estimation_indices, closest_subcarrier):
    from concourse.bass_utils import run_bass_kernel_spmd

    assert y_real.shape == (B, R, A, T, S), y_real.shape
    est = np.asarray(estimation_indices)
    closest = np.asarray(closest_subcarrier)
    kind, nc, wt = _get_program(est, closest)

    yr = np.ascontiguousarray(np.asarray(y_real, dtype=np.float32)[:, 0])
    yi = np.ascontiguousarray(np.asarray(y_imag, dtype=np.float32)[:, 0])

    if kind == "fast":
        in_maps = [
            {
                "yr": yr[c * B_LOC : (c + 1) * B_LOC],
                "yi": yi[c * B_LOC : (c + 1) * B_LOC],
            }
            for c in range(N_CORES)
        ]
    else:
        sym = est[:, 0].astype(np.int64)
        sc = est[:, 1].astype(np.int64)
        # host gather: sig[b, a, p] = y[b, a, sym_p, sc_p]
        sgr = yr[:, :, sym, sc]  # [B, A, P]
        sgi = yi[:, :, sym, sc]
        # -> [B, KP, 128, A]
        sgr = np.ascontiguousarray(
            sgr.transpose(0, 2, 1).reshape(B, P_EST // 128, 128, A)
        )
        sgi = np.ascontiguousarray(
            sgi.transpose(0, 2, 1).reshape(B, P_EST // 128, 128, A)
        )
        in_maps = [
            {
                "sgr": sgr[c * B_LOC : (c + 1) * B_LOC],
                "sgi": sgi[c * B_LOC : (c + 1) * B_LOC],
                "wt": wt,
            }
            for c in range(N_CORES)
        ]

    res = run_bass_kernel_spmd(nc, in_maps, list(range(N_CORES)))
    parts = [res.results[c]["out"] for c in range(N_CORES)]
    full = np.concatenate(parts, axis=0)  # [B, T, S, AA2]
    return full.view(np.complex64).reshape(B, R, T, S, A, A)



# revision 11
# speedup vs baseline: 1.0794x; 1.0794x over previous
"""Trainium2 Bass kernel for nn_CovarianceEstimator.

Computes, for y [B=16, R=1, A=16, T=14, S=1024] complex (given as separate
real/imag f32 tensors):
  - gather P=1024 pilot positions (sym_p, sc_p) from estimation_indices
  - per-position A x A outer products sig_p sig_p^H
  - unsorted-segment-mean over subcarrier ids sc_p
  - nearest-neighbor expand via closest_subcarrier to all S subcarriers
  - broadcast over T symbols
Output: [B, R, T, S, A, A] complex64.

Sharding: data-parallel over batch; 2 batches per core on 8 cores.

The kernel is HBM-write-bound: the per-core output is 58.7 MB and streams
at the ~427 GB/s per-core DMA ceiling (~137 us).  The design minimizes the
serial prefix before the output stream starts:
  - one descriptor-efficient bulk load of the two pilot-symbol slabs
    (partition = (batch, re/im, sym, antenna), 4 KB contiguous runs)
  - PE transpose (matmul against a sqrt(1/2)-scaled identity, which also
    applies the segment-mean factor) to put subcarriers into partitions
  - DVE outer products in 4 chunks per batch, each followed immediately by
    its slice of the output DMA; nearest-neighbor row duplication and the
    T-broadcast are stride-0 dims of the DMA source pattern, so each cov
    element is computed once and fanned out by the DMA engines.

Two device-program builders:
  * fast path  - used when the index tensors match the PilotPattern structure
                 (meshgrid of 2 symbols x every-2nd-subcarrier, closest = even
                 floor).  Exact f32 math.
  * generic    - any estimation_indices / closest_subcarrier.  Host folds the
                 whole segment-mean + NN-gather into one dense [S, P] weight
                 matrix applied on the tensor engine.
"""

import numpy as np

B, R, A, T, S = 16, 1, 16, 14, 1024
P_EST = 1024          # number of (sym, sc) estimation positions
N_CORES = 8
B_LOC = B // N_CORES  # 2 batches per core
AA2 = A * A * 2       # interleaved (re, im) row payload per subcarrier

_cache = {}


def _fast_path_info(est, closest):
    """Return (sym0, sym1) if indices match the pilot-pattern structure:
    est == meshgrid([sym0, sym1], arange(0, S, 2)) row-major and
    closest == 2*(arange(S)//2).  Else None."""
    if est.shape != (P_EST, 2) or closest.shape != (S,):
        return None
    sc = np.arange(0, S, 2, dtype=est.dtype)
    if not np.array_equal(est[: S // 2, 1], sc):
        return None
    if not np.array_equal(est[S // 2 :, 1], sc):
        return None
    sym0 = int(est[0, 0])
    sym1 = int(est[S // 2, 0])
    if not (0 <= sym0 < T and 0 <= sym1 < T):
        return None
    if sym1 <= sym0:
        return None  # strided 2-element AP needs sym1 > sym0
    if not np.all(est[: S // 2, 0] == sym0):
        return None
    if not np.all(est[S // 2 :, 0] == sym1):
        return None
    if not np.array_equal(closest, (2 * (np.arange(S) // 2)).astype(closest.dtype)):
        return None
    return sym0, sym1


def _build_fast(sym0, sym1):
    """Fast-path device program.  Pipeline per batch b and chunk m (4 chunks,
    one per even-subcarrier pair in a partition):
      psT[b][m][q, (ri h a)] = sqrt(1/2) * y[b, ., sym_h, 8q + 2m]   (PE)
      fd[b][m][q, i*A+j, ri] = cov(s' = 4q + m)                      (DVE)
      out[b, t, 8q + 2m + e, :] = fd[b][m][q]   for all t, e         (DMA)
    The DMA source uses stride-0 dims for both t (symbol broadcast) and e
    (nearest-neighbor row duplication)."""
    import concourse.bacc as bacc
    import concourse.mybir as mybir
    from concourse.tile import TileContext

    f32 = mybir.dt.float32
    nc = bacc.Bacc(trn_type="TRN2", target_bir_lowering=False)
    yr = nc.declare_dram_parameter("yr", [B_LOC, A, T, S], f32, isOutput=False)
    yi = nc.declare_dram_parameter("yi", [B_LOC, A, T, S], f32, isOutput=False)
    out = nc.declare_dram_parameter("out", [B_LOC, T, S, AA2], f32, isOutput=True)

    KS = S // 128   # 8 output subcarriers per partition
    M = KS // 2     # 4 even-subcarrier pairs per partition
    HA = 2 * A      # (h, a) = 32
    C = 2 * HA      # (ri, h, a) = 64 bulk partitions per batch
    step = sym1 - sym0

    with TileContext(nc) as tc:
        with (
            tc.tile_pool(name="const", bufs=1) as cp,
            tc.tile_pool(name="bulk", bufs=1) as bp,
            tc.tile_pool(name="ps", bufs=1, space="PSUM") as psp,
            tc.tile_pool(name="u", bufs=2) as up,
            tc.tile_pool(name="f", bufs=1) as fp,
        ):
            # sqrt(1/2)-scaled identity: the PE transpose then applies the
            # segment-mean 1/2 to every product for free.
            ident = cp.tile([C, C], f32, name="ident")
            nc.gpsimd.memset(ident[:], 0.0)
            nc.gpsimd.affine_select(
                out=ident[:],
                in_=ident[:],
                compare_op=mybir.AluOpType.not_equal,
                fill=0.7071067811865476,
                base=0,
                pattern=[[-1, C]],
                channel_multiplier=1,
            )

            # Bulk pilot slabs: partition p = ri*32 + h*16 + a per batch,
            # each partition one contiguous 4 KB DRAM run.
            bulk = [bp.tile([C, S], f32, name=f"bulk{b}") for b in range(B_LOC)]
            for b in range(B_LOC):
                for ri, (ysrc, eng) in enumerate(((yr, nc.sync), (yi, nc.scalar))):
                    for h, sym in enumerate((sym0, sym1)):
                        p0 = ri * HA + h * A
                        eng.dma_start(
                            out=bulk[b][p0 : p0 + A],
                            in_=ysrc[b, :, sym, :],
                        )

            psT = [
                [psp.tile([128, C], f32, tag=f"ps{b}{m}", name=f"ps{b}{m}") for m in range(M)]
                for b in range(B_LOC)
            ]
            fdt = [
                [fp.tile([128, 2, A * A, 2], f32, tag=f"fd{b}{m}", name=f"fd{b}{m}") for m in range(M)]
                for b in range(B_LOC)
            ]

            sig = [
                [cp.tile([128, C], f32, tag=f"sig{b}{m}", name=f"sig{b}{m}") for m in range(M)]
                for b in range(B_LOC)
            ]

            for b in range(B_LOC):
                # PE transpose: even subcarrier s = 8q + 2m into partition q,
                # then ACT copies PSUM -> SBUF (DVE can read only one PSUM
                # operand per instruction, and the outer products need two).
                for m in range(M):
                    nc.tensor.matmul(
                        psT[b][m][:],
                        lhsT=bulk[b][:, 2 * m :: KS],
                        rhs=ident[:],
                        start=True,
                        stop=True,
                    )
                    nc.scalar.copy(sig[b][m][:], psT[b][m][:])

                for m in range(M):
                    sr = sig[b][m][:, 0:HA].rearrange("q (h a) -> q h a", h=2)
                    si = sig[b][m][:, HA:C].rearrange("q (h a) -> q h a", h=2)

                    def vi(x):  # varies over i, broadcast over j
                        return x[:, :, :, None].to_broadcast([128, 2, A, A])

                    def vj(x):  # broadcast over i, varies over j
                        return x[:, :, None, :].to_broadcast([128, 2, A, A])

                    # fd holds both duplicate rows e = 0, 1 (nearest-neighbor
                    # expand); the final adds write them in one op via an
                    # e-broadcast destination.
                    fd = fdt[b][m]
                    fre = fd[:, :, :, 0].rearrange("q e (i j) -> q e i j", i=A)
                    fim = fd[:, :, :, 1].rearrange("q e (i j) -> q e i j", i=A)
                    u0 = up.tile([128, 2, A, A], f32, tag="u0")
                    u1 = up.tile([128, 2, A, A], f32, tag="u1")
                    w0 = up.tile([128, 2, A, A], f32, tag="w0")
                    w1 = up.tile([128, 2, A, A], f32, tag="w1")

                    def ve(x):  # broadcast an [q, A, A] term over e
                        return x[:, None, :, :].to_broadcast([128, 2, A, A])

                    # real: sum_h SrSr + SiSi
                    nc.vector.tensor_mul(u0[:], vi(sr), vj(sr))
                    nc.vector.tensor_mul(u1[:], vi(si), vj(si))
                    nc.vector.tensor_add(w0[:], u0[:], u1[:])
                    nc.vector.tensor_add(fre, ve(w0[:, 0]), ve(w0[:, 1]))
                    # imag: sum_h SiSr - SrSi
                    nc.vector.tensor_mul(u0[:], vi(si), vj(sr))
                    nc.vector.tensor_mul(u1[:], vi(sr), vj(si))
                    nc.vector.tensor_sub(w1[:], u0[:], u1[:])
                    nc.vector.tensor_add(fim, ve(w1[:, 0]), ve(w1[:, 1]))

                    # Output rows s = 8q + 2m + e for all t: one DMA per
                    # (b, m); t is a stride-0 dim of the source.
                    src = (
                        fd[:]
                        .rearrange("q e c ri -> q (e c ri)")[:, None, :]
                        .to_broadcast([128, T, 2 * A * A * 2])
                    )
                    dst = out[b].rearrange("t (q k) c -> q t (k c)", q=128, k=KS)[
                        :, :, 2 * m * AA2 : (2 * m + 2) * AA2
                    ]
                    nc.sync.dma_start(out=dst, in_=src)
    nc.finalize()
    return nc


def _build_generic(est, closest):
    """Generic program: host-gathered sig^T comes in as an input; the whole
    segment-mean + NN-gather is one dense weight matmul on the PE.
      cov[s, (i,j)] = sum_p wt[p, s] * G[p, (i,j)],  G from sig outer products.
    """
    import concourse.bacc as bacc
    import concourse.mybir as mybir
    from concourse.tile import TileContext

    f32 = mybir.dt.float32
    nc = bacc.Bacc(trn_type="TRN2", target_bir_lowering=False)
    # sig^T per batch: [P_EST, A] split as [KP=8, 128, A]
    sgr = nc.declare_dram_parameter("sgr", [B_LOC, P_EST // 128, 128, A], f32, isOutput=False)
    sgi = nc.declare_dram_parameter("sgi", [B_LOC, P_EST // 128, 128, A], f32, isOutput=False)
    wt = nc.declare_dram_parameter("wt", [P_EST, S], f32, isOutput=False)
    out = nc.declare_dram_parameter("out", [B_LOC, T, S, AA2], f32, isOutput=True)

    KP = P_EST // 128  # contraction chunks
    MS = S // 128      # output subcarrier chunks

    with TileContext(nc) as tc:
        with (
            tc.tile_pool(name="w", bufs=1) as wp,
            tc.tile_pool(name="sig", bufs=2) as sigp,
            tc.tile_pool(name="g", bufs=4) as gp,
            tc.tile_pool(name="ps", bufs=8, space="PSUM") as psp,
            tc.tile_pool(name="f", bufs=2) as fp,
        ):
            w_all = wp.tile([128, KP, S], f32, name="w_all")
            nc.sync.dma_start(
                out=w_all[:], in_=wt[:].rearrange("(k q) s -> q k s", k=KP, q=128)
            )
            for b in range(B_LOC):
                sr = sigp.tile([128, KP, A], f32, tag="sr")
                si = sigp.tile([128, KP, A], f32, tag="si")
                nc.sync.dma_start(
                    out=sr[:], in_=sgr[b].rearrange("k q a -> q k a")
                )
                nc.sync.dma_start(
                    out=si[:], in_=sgi[b].rearrange("k q a -> q k a")
                )

                f = fp.tile([128, MS, A * A, 2], f32, tag="f")
                gtiles = {}
                for k in range(KP):
                    def ii(x):
                        return x[:, k, :, None].to_broadcast([128, A, A])

                    def jj(x):
                        return x[:, k, None, :].to_broadcast([128, A, A])

                    gr = gp.tile([128, A, A], f32, tag=f"gr{k}")
                    gi = gp.tile([128, A, A], f32, tag=f"gi{k}")
                    tt = gp.tile([128, A, A], f32, tag="tt")
                    nc.vector.tensor_mul(gr[:], ii(sr), jj(sr))
                    nc.vector.tensor_mul(tt[:], ii(si), jj(si))
                    nc.vector.tensor_add(gr[:], gr[:], tt[:])
                    nc.vector.tensor_mul(gi[:], ii(si), jj(sr))
                    nc.vector.tensor_mul(tt[:], ii(sr), jj(si))
                    nc.vector.tensor_sub(gi[:], gi[:], tt[:])
                    gtiles[k] = (gr, gi)

                for m in range(MS):
                    for part in range(2):
                        pp = psp.tile([128, A * A], f32, tag="pp")
                        for k in range(KP):
                            g = gtiles[k][part]
                            nc.tensor.matmul(
                                pp[:],
                                lhsT=w_all[:, k, m * 128 : (m + 1) * 128],
                                rhs=g[:].rearrange("q i j -> q (i j)"),
                                start=(k == 0),
                                stop=(k == KP - 1),
                            )
                        nc.vector.tensor_copy(f[:, m, :, part], pp[:])

                src = f[:]
                for t in range(T):
                    dst = out[b, t].rearrange(
                        "(m q) (ij ri) -> q m ij ri", m=MS, q=128, ij=A * A, ri=2
                    )
                    nc.sync.dma_start(out=dst, in_=src)
    nc.finalize()
    return nc


def _get_program(est, closest):
    key = (est.tobytes(), closest.tobytes())
    hit = _cache.get(key)
    if hit is not None:
        return hit
    fast = _fast_path_info(est, closest)
    if fast is not None:
        prog = ("fast", _build_fast(*fast), None)
    else:
        counts = np.zeros(S, dtype=np.float64)
        np.add.at(counts, est[:, 1], 1.0)
        denom = np.maximum(counts, 1.0)
        # wt[p, s] = [sc_p == closest[s]] / denom[closest[s]]
        wt = (
            (est[:, 1][:, None] == closest[None, :]).astype(np.float32)
            / denom[closest][None, :].astype(np.float32)
        )
        prog = ("generic", _build_generic(est, closest), np.ascontiguousarray(wt))
    _cache[key] = prog
    return prog


def kernel(y_real, y_imag, estimation_indices, closest_subcarrier):
    from concourse.bass_utils import run_bass_kernel_spmd

    assert y_real.shape == (B, R, A, T, S), y_real.shape
    est = np.asarray(estimation_indices)
    closest = np.asarray(closest_subcarrier)
    kind, nc, wt = _get_program(est, closest)

    yr = np.ascontiguousarray(np.asarray(y_real, dtype=np.float32)[:, 0])
    yi = np.ascontiguousarray(np.asarray(y_imag, dtype=np.float32)[:, 0])

    if kind == "fast":
        in_maps = [
            {
                "yr": yr[c * B_LOC : (c + 1) * B_LOC],
                "yi": yi[c * B_LOC : (c + 1) * B_LOC],
            }
            for c in range(N_CORES)
        ]
    else:
        sym = est[:, 0].astype(np.int64)
        sc = est[:, 1].astype(np.int64)
        # host gather: sig[b, a, p] = y[b, a, sym_p, sc_p]
        sgr = yr[:, :, sym, sc]  # [B, A, P]
        sgi = yi[:, :, sym, sc]
        # -> [B, KP, 128, A]
        sgr = np.ascontiguousarray(
            sgr.transpose(0, 2, 1).reshape(B, P_EST // 128, 128, A)
        )
        sgi = np.ascontiguousarray(
            sgi.transpose(0, 2, 1).reshape(B, P_EST // 128, 128, A)
        )
        in_maps = [
            {
                "sgr": sgr[c * B_LOC : (c + 1) * B_LOC],
                "sgi": sgi[c * B_LOC : (c + 1) * B_LOC],
                "wt": wt,
            }
            for c in range(N_CORES)
        ]

    res = run_bass_kernel_spmd(nc, in_maps, list(range(N_CORES)))
    parts = [res.results[c]["out"] for c in range(N_CORES)]
    full = np.concatenate(parts, axis=0)  # [B, T, S, AA2]
    return full.view(np.complex64).reshape(B, R, T, S, A, A)
